# revision 40
# baseline (speedup 1.0000x reference)
"""Trainium2 Bass kernel for the fused candidate-attention module.

Computation (reference, fp32):
    delta[n,l,m] = sum_k self_delta[n,m,l,k]
    out[n,l]     = sum_m value_w[m] * delta[n,l,m] * (emb[1+l,:] . self_attn[n,m,:])

Sharding: data-parallel over batch N (4 batches per core across 8 cores);
emb table and value weight are replicated.  Host-side prep (not on the
device critical path): self_delta is cast to bf16 and k-deinterleaved to
[N, M, K, L]; value_w is folded into self_attn (w = vw * attn, bf16); the
candidate embedding slice is pre-transposed to [D, L].

Per-core device pipeline, per batch n (L = 8192 candidates, chunks of 1024):

    dd   [m=100, 2*8192]  <- one 3.2 MB HWDGE DMA (declared f32, bitcast bf16)
    g    [d=128, 1024]    = w_n^T @ dd_k0 + w_n^T @ dd_k1  (PE, bf16 PSUM)
    prod [d=128, 1024]    = g * embT[:, chunk]             (DVE, direct PSUM)
    row  [32j:, 1024]     = ones32^T @ prod                (PE, ones stationary,
                            4 chunks pack one PSUM bank via tile_position;
                            rows are 32x duplicated so the evict is dense)
    out_sb                <- one ACT bank copy per 4 chunks (2 per batch)
    out DMA               <- one 32 KB SWDGE store per batch
"""

import os
from contextlib import ExitStack

import numpy as np
import ml_dtypes

import concourse.bacc as bacc
import concourse.bass as bass
import concourse.mybir as mybir
from concourse.bass_utils import run_bass_kernel_spmd
from concourse.tile import TileContext

N, M, L, K, D = 32, 100, 8192, 2, 128
NCORES = 8
NB = N // NCORES  # batches per core

F32 = mybir.dt.float32
BF16 = mybir.dt.bfloat16
F16 = mybir.dt.float16

NPBF16 = ml_dtypes.bfloat16

DDBUFS = int(os.environ.get("KERNEL_DDBUFS", "3"))
# "bf16": 1024-wide chunks, bf16 PSUM, DVE reads PSUM directly.
# "f32": 512-wide chunks, fp32 PSUM (fallback if bf16 PSUM accum is broken).
GDT = os.environ.get("KERNEL_GDT", "f32")
# 0 = all chunks multiply straight from PSUM on DVE; k>0 = every k-th chunk
# goes through an ACT f16-evict + DVE 2x multiply instead (engine balancing)
HYB = int(os.environ.get("KERNEL_HYB", "0"))
# per-batch delta-DMA engine map: s=sync(SP), a=scalar(ACT), g=gpsimd(SWDGE)
DENG = os.environ.get("KERNEL_DENG", "ssss")
EMBENG = os.environ.get("KERNEL_EMBENG", "a")
# 1 = sum self_delta over k on the host (halves HBM traffic and PE passes)
K1 = int(os.environ.get("KERNEL_K1", "0"))
KEFF = 1 if K1 else K
# Benchmarking only: device-side repeat of the main loop (1 = no loop)
LOOP_R = int(os.environ.get("KERNEL_LOOP", "1"))
# Benchmarking only: pipeline prefix to build ("dma", "mm", "mul", "red", "full")
STAGE = os.environ.get("KERNEL_STAGE", "full")

MMF = 1024 if GDT == "bf16" else 512
NCHUNK = L // MMF
GRP = 4  # chunks per reduce group (4 tile positions in one PSUM bank)
NGRPS = NCHUNK // GRP


def _build_nc() -> bass.Bass:
    g_dt = BF16 if GDT == "bf16" else F32
    emb_dt = BF16 if GDT == "bf16" else F32
    prod_dt = BF16 if GDT == "bf16" else F16
    nc = bacc.Bacc()

    # delta bytes are bf16 [NB, M, KEFF, L] but declared f32 so the DMA moves
    # 4-byte elements (2-byte-dtype DMAs run at half port rate)
    delta = nc.declare_dram_parameter(
        "delta", [NB, M, KEFF * L // 2], F32, isOutput=False
    )
    wp = nc.declare_dram_parameter("w", [M, NB * D], BF16, isOutput=False)
    embp = nc.declare_dram_parameter("embt", [D, L], emb_dt, isOutput=False)
    emb16p = (
        nc.declare_dram_parameter("embt16", [D, L], F16, isOutput=False)
        if HYB
        else None
    )
    outp = nc.declare_dram_parameter("out", [NB, L], F32, isOutput=True)

    with TileContext(nc) as tc, ExitStack() as ctx:
        const = ctx.enter_context(tc.tile_pool(name="const", bufs=1))

        _engs = {"s": nc.sync, "a": nc.scalar, "g": nc.gpsimd}
        embT = const.tile([D, L], emb_dt)
        _engs[EMBENG].dma_start(out=embT[:], in_=embp[:])

        w_sb = const.tile([M, NB * D], BF16)
        nc.scalar.dma_start(out=w_sb[:], in_=wp[:])

        embT16 = None
        if HYB:
            embT16 = const.tile([D, L], F16)
            nc.scalar.dma_start(out=embT16[:], in_=emb16p[:])

        # 32 identical columns: the reduce duplicates each chunk-row across a
        # full 32-partition group so the PSUM evict is a dense bank copy
        ones = const.tile([D, 32], prod_dt)
        nc.vector.memset(ones[:], 1.0)

        # out staging: one [128, MMF] bank image per (batch, group); partition
        # 32*j holds the row of chunk grp*GRP+j (rows are 32x duplicated)
        out_sb = const.tile([128, NB * NGRPS * MMF], F32)

        dd_pool = ctx.enter_context(tc.tile_pool(name="dd", bufs=DDBUFS))
        gs_pool = (
            ctx.enter_context(tc.tile_pool(name="gs", bufs=4)) if HYB else None
        )
        g_psum = ctx.enter_context(tc.tile_pool(name="g", bufs=4, space="PSUM"))
        prod_pool = ctx.enter_context(tc.tile_pool(name="prod", bufs=2 * GRP + 1))
        row_psum = ctx.enter_context(tc.tile_pool(name="row", bufs=2, space="PSUM"))

        loop_ctx = tc.For_i(0, LOOP_R, 1) if LOOP_R > 1 else None
        if loop_ctx is not None:
            ctx.enter_context(loop_ctx)

        pending = []  # (n, grp, j, prod) awaiting the partition-reduce
        done_groups: dict[int, int] = {}
        for n in range(NB):
            W = KEFF * L // 2  # f32 cols per batch
            dd = dd_pool.tile([M, W], F32)
            deng = _engs[DENG[n % len(DENG)]]
            if n == 0:
                # batch 0 arrives in pieces (one per k-half x l-quarter,
                # interleaved) so the PE can start ~2.5us in instead of
                # waiting for the whole transfer
                q = W // (4 * KEFF)  # f32 cols per piece
                for r in range(4):
                    for kk in range(KEFF):
                        c0 = kk * (W // KEFF) + r * q
                        deng.dma_start(
                            out=dd[:, c0 : c0 + q], in_=delta[n, :, c0 : c0 + q]
                        )
            else:
                # one contiguous DMA per batch; ring per DENG map
                deng.dma_start(out=dd[:], in_=delta[n])
            ddv = dd[:].bitcast(BF16)  # [M, KEFF*L] bf16 view
            if STAGE == "dma":
                continue
            w_n = w_sb[:, n * D : (n + 1) * D]
            for grp in range(NGRPS):
                for j in range(GRP):
                    h = grp * GRP + j
                    ls = slice(h * MMF, (h + 1) * MMF)
                    g = g_psum.tile([D, MMF], g_dt)
                    for kk in range(KEFF):
                        nc.tensor.matmul(
                            g[:],
                            lhsT=w_n,
                            rhs=ddv[:, kk * L + h * MMF : kk * L + (h + 1) * MMF],
                            start=(kk == 0),
                            stop=(kk == KEFF - 1),
                        )
                    if STAGE == "mm":
                        continue
                    prod = prod_pool.tile([D, MMF], prod_dt)
                    if HYB and h % HYB == 0:
                        # ACT evicts PSUM as f16 so the DVE multiply runs in
                        # its 2x packed mode — balances DVE vs ACT load
                        gs = gs_pool.tile([D, MMF], F16)
                        nc.scalar.copy(gs[:], g[:])
                        nc.vector.tensor_mul(prod[:], gs[:], embT16[:, ls])
                    else:
                        nc.vector.tensor_mul(prod[:], g[:], embT[:, ls])
                    if STAGE in ("red", "full"):
                        pending.append((n, grp, j, prod))
                # drain the PREVIOUS group's reduces so the PE never waits on
                # the DVE chain of the group it just issued
                while len(pending) > GRP or (
                    pending and (n, grp) == (NB - 1, NGRPS - 1)
                ):
                    flush = pending[:GRP]
                    del pending[:GRP]
                    row = row_psum.tile([128, MMF], g_dt, tag="row")
                    for pn, pgrp, pj, prod in flush:
                        # partition-reduce over d: ones stationary (32 dup
                        # cols); chunk pj lands at partitions [32pj, 32pj+32)
                        nc.tensor.matmul(
                            row[32 * pj : 32 * pj + 32, :], lhsT=ones[:],
                            rhs=prod[:], start=True, stop=True,
                            tile_position=(0, 32 * pj),
                        )
                    pn, pgrp = flush[0][0], flush[0][1]
                    gcol = (pn * NGRPS + pgrp) * MMF
                    nc.scalar.copy(out_sb[:, gcol : gcol + MMF], row[:])
                    done_groups[pn] = done_groups.get(pn, 0) + 1
                    if STAGE == "full" and done_groups[pn] == NGRPS:
                        # store batch pn on SWDGE so the HWDGE load FIFO
                        # never queues a store behind the next delta load
                        nc.gpsimd.dma_start(
                            out=outp[pn].rearrange(
                                "(grp j f) -> j grp f", j=GRP, f=MMF
                            ),
                            in_=out_sb[
                                :, pn * NGRPS * MMF : (pn + 1) * NGRPS * MMF
                            ].rearrange(
                                "(j d) (grp f) -> j d grp f", d=32, f=MMF
                            )[:, 0],
                        )

    nc.compile()
    return nc


_NC_CACHE: dict[str, bass.Bass] = {}


def _get_nc() -> bass.Bass:
    key = f"{LOOP_R}:{STAGE}:{DDBUFS}:{GDT}:{HYB}:{DENG}:{EMBENG}:{K1}"
    if key not in _NC_CACHE:
        _NC_CACHE[key] = _build_nc()
    return _NC_CACHE[key]


def prepare_in_maps(self_attn, self_delta, emb_table, value_w):
    """Host-side prep: cast/fold/transpose + shard over batch."""
    self_attn = np.asarray(self_attn, dtype=np.float32)
    self_delta = np.asarray(self_delta, dtype=np.float32)
    emb_table = np.asarray(emb_table, dtype=np.float32)
    value_w = np.asarray(value_w, dtype=np.float32)
    assert self_attn.shape == (N, M, D), self_attn.shape
    assert self_delta.shape == (N, M, L, K), self_delta.shape
    assert emb_table.shape == (L + 1, D), emb_table.shape

    if K1:
        # host k-sum: [N, M, L, K] f32 -> [N, M, L] bf16 -> f32 view
        dkl = (
            (self_delta[..., 0] + self_delta[..., 1])
            .astype(NPBF16)
            .view(np.float32)
            .reshape(N, M, L // 2)
        )
    else:
        # [N, M, L, K] f32 -> [N, M, K, L] bf16, then f32 view [N, M, K*L/2]
        dkl = np.ascontiguousarray(
            self_delta.astype(NPBF16).transpose(0, 1, 3, 2)
        ).view(np.float32).reshape(N, M, K * L // 2)
    w = (self_attn * value_w[None, :, None]).astype(NPBF16)  # [N, M, D]
    embt32 = np.ascontiguousarray(emb_table[1:].T)  # [D, L] f32
    embt = embt32.astype(NPBF16) if GDT == "bf16" else embt32

    in_maps = []
    for c in range(NCORES):
        n0 = c * NB
        m = {
            "delta": dkl[n0 : n0 + NB],
            "w": np.ascontiguousarray(
                w[n0 : n0 + NB].transpose(1, 0, 2)
            ).reshape(M, NB * D),
            "embt": embt,
        }
        if HYB:
            m["embt16"] = embt32.astype(np.float16)
        in_maps.append(m)
    return in_maps


def kernel(self_attn, self_delta, emb_table, value_w, traj_len=None, loc_max=None,
           _trace=False, _tmpdir=None):
    """Full inputs in, full output out.  traj_len is unused by the reference."""
    if loc_max is not None:
        assert int(loc_max) == L, loc_max

    in_maps = prepare_in_maps(self_attn, self_delta, emb_table, value_w)

    nc = _get_nc()
    try:
        res = run_bass_kernel_spmd(
            nc, in_maps, list(range(NCORES)), trace=_trace, tmpdir=_tmpdir
        )
    except Exception:
        # one retry for transient NRT execution failures
        res = run_bass_kernel_spmd(
            nc, in_maps, list(range(NCORES)), trace=_trace, tmpdir=_tmpdir
        )
    out = np.concatenate([res.results[c]["out"] for c in range(NCORES)], axis=0)
    if _trace:
        return out, res
    return out


# revision 42
# speedup vs baseline: 1.8638x; 1.8638x over previous
"""Trainium2 Bass kernel for the fused candidate-attention module.

Computation (reference, fp32):
    delta[n,l,m] = sum_k self_delta[n,m,l,k]
    out[n,l]     = sum_m value_w[m] * delta[n,l,m] * (emb[1+l,:] . self_attn[n,m,:])

Sharding: data-parallel over batch N (4 batches per core across 8 cores);
emb table and value weight are replicated.  Host-side prep (not on the
device critical path): self_delta is cast to bf16 and k-deinterleaved to
[N, M, K, L]; value_w is folded into self_attn (w = vw * attn, bf16); the
candidate embedding slice is pre-transposed to [D, L].

Per-core device pipeline, per batch n (L = 8192 candidates, chunks of 1024):

    dd   [m=100, 2*8192]  <- one 3.2 MB HWDGE DMA (declared f32, bitcast bf16)
    g    [d=128, 1024]    = w_n^T @ dd_k0 + w_n^T @ dd_k1  (PE, bf16 PSUM)
    prod [d=128, 1024]    = g * embT[:, chunk]             (DVE, direct PSUM)
    row  [32j:, 1024]     = ones32^T @ prod                (PE, ones stationary,
                            4 chunks pack one PSUM bank via tile_position;
                            rows are 32x duplicated so the evict is dense)
    out_sb                <- one ACT bank copy per 4 chunks (2 per batch)
    out DMA               <- one 32 KB SWDGE store per batch
"""

import os
from contextlib import ExitStack

import numpy as np
import ml_dtypes

import concourse.bacc as bacc
import concourse.bass as bass
import concourse.mybir as mybir
from concourse.bass_utils import run_bass_kernel_spmd
from concourse.tile import TileContext

N, M, L, K, D = 32, 100, 8192, 2, 128
NCORES = 8
NB = N // NCORES  # batches per core

F32 = mybir.dt.float32
BF16 = mybir.dt.bfloat16
F16 = mybir.dt.float16

NPBF16 = ml_dtypes.bfloat16

DDBUFS = int(os.environ.get("KERNEL_DDBUFS", "3"))
# "bf16": 1024-wide chunks, bf16 PSUM, DVE reads PSUM directly.
# "f32": 512-wide chunks, fp32 PSUM (fallback if bf16 PSUM accum is broken).
GDT = os.environ.get("KERNEL_GDT", "f32")
# 0 = all chunks multiply straight from PSUM on DVE; k>0 = every k-th chunk
# goes through an ACT f16-evict + DVE 2x multiply instead (engine balancing)
HYB = int(os.environ.get("KERNEL_HYB", "0"))
# per-batch delta-DMA engine map: s=sync(SP), a=scalar(ACT), g=gpsimd(SWDGE)
DENG = os.environ.get("KERNEL_DENG", "ssss")
EMBENG = os.environ.get("KERNEL_EMBENG", "a")
# 1 = sum self_delta over k on the host (halves HBM traffic and PE passes)
K1 = int(os.environ.get("KERNEL_K1", "0"))
KEFF = 1 if K1 else K
# per-batch DMA piece counts (cycled) and ring-alternation policy
PIECES = [int(x) for x in os.environ.get("KERNEL_PIECES", "8,1,1,1").split(",")]
RALT = int(os.environ.get("KERNEL_RALT", "0"))
# Benchmarking only: device-side repeat of the main loop (1 = no loop)
LOOP_R = int(os.environ.get("KERNEL_LOOP", "1"))
# Benchmarking only: pipeline prefix to build ("dma", "mm", "mul", "red", "full")
STAGE = os.environ.get("KERNEL_STAGE", "full")

MMF = 1024 if GDT == "bf16" else 512
NCHUNK = L // MMF
GRP = 4  # chunks per reduce group (4 tile positions in one PSUM bank)
NGRPS = NCHUNK // GRP


def _build_nc() -> bass.Bass:
    g_dt = BF16 if GDT == "bf16" else F32
    emb_dt = BF16 if GDT == "bf16" else F32
    prod_dt = BF16 if GDT == "bf16" else F16
    nc = bacc.Bacc()

    # delta bytes are bf16 [NB, M, KEFF, L] but declared f32 so the DMA moves
    # 4-byte elements (2-byte-dtype DMAs run at half port rate)
    delta = nc.declare_dram_parameter(
        "delta", [NB, M, KEFF * L // 2], F32, isOutput=False
    )
    wp = nc.declare_dram_parameter("w", [M, NB * D], BF16, isOutput=False)
    embp = nc.declare_dram_parameter("embt", [D, L], emb_dt, isOutput=False)
    emb16p = (
        nc.declare_dram_parameter("embt16", [D, L], F16, isOutput=False)
        if HYB
        else None
    )
    outp = nc.declare_dram_parameter("out", [NB, L], F32, isOutput=True)

    with TileContext(nc) as tc, ExitStack() as ctx:
        const = ctx.enter_context(tc.tile_pool(name="const", bufs=1))

        _engs = {"s": nc.sync, "a": nc.scalar, "g": nc.gpsimd}
        embT = const.tile([D, L], emb_dt)
        _engs[EMBENG].dma_start(out=embT[:], in_=embp[:])

        w_sb = const.tile([M, NB * D], BF16)
        nc.scalar.dma_start(out=w_sb[:], in_=wp[:])

        embT16 = None
        if HYB:
            embT16 = const.tile([D, L], F16)
            nc.scalar.dma_start(out=embT16[:], in_=emb16p[:])

        # 32 identical columns: the reduce duplicates each chunk-row across a
        # full 32-partition group so the PSUM evict is a dense bank copy
        ones = const.tile([D, 32], prod_dt)
        nc.vector.memset(ones[:], 1.0)

        # out staging: one [128, MMF] bank image per (batch, group); partition
        # 32*j holds the row of chunk grp*GRP+j (rows are 32x duplicated)
        out_sb = const.tile([128, NB * NGRPS * MMF], F32)

        dd_pool = ctx.enter_context(tc.tile_pool(name="dd", bufs=DDBUFS))
        gs_pool = (
            ctx.enter_context(tc.tile_pool(name="gs", bufs=4)) if HYB else None
        )
        g_psum = ctx.enter_context(tc.tile_pool(name="g", bufs=4, space="PSUM"))
        prod_pool = ctx.enter_context(tc.tile_pool(name="prod", bufs=2 * GRP + 1))
        row_psum = ctx.enter_context(tc.tile_pool(name="row", bufs=2, space="PSUM"))

        loop_ctx = tc.For_i(0, LOOP_R, 1) if LOOP_R > 1 else None
        if loop_ctx is not None:
            ctx.enter_context(loop_ctx)

        pending = []  # (n, grp, j, prod) awaiting the partition-reduce
        done_groups: dict[int, int] = {}
        for n in range(NB):
            W = KEFF * L // 2  # f32 cols per batch
            dd = dd_pool.tile([M, W], F32)
            # Pieces arrive k-interleaved ((k0,q0),(k1,q0),(k0,q1),...) so the
            # PE can start on the first l-quarter instead of waiting for the
            # whole batch; consecutive pieces alternate the two HWDGE rings,
            # which leapfrogs the per-DMA completion latency.
            np_ = PIECES[n % len(PIECES)]
            nq = max(1, np_ // KEFF)
            q = W // (nq * KEFF)  # f32 cols per piece
            for r in range(nq):
                for kk in range(KEFF):
                    c0 = kk * (W // KEFF) + r * q
                    if RALT:
                        deng = (nc.sync, nc.scalar)[_dma_ctr[0] % 2]
                        _dma_ctr[0] += 1
                    else:
                        deng = _engs[DENG[n % len(DENG)]]
                    deng.dma_start(
                        out=dd[:, c0 : c0 + q], in_=delta[n, :, c0 : c0 + q]
                    )
            ddv = dd[:].bitcast(BF16)  # [M, KEFF*L] bf16 view
            if STAGE == "dma":
                continue
            w_n = w_sb[:, n * D : (n + 1) * D]
            for grp in range(NGRPS):
                for j in range(GRP):
                    h = grp * GRP + j
                    ls = slice(h * MMF, (h + 1) * MMF)
                    g = g_psum.tile([D, MMF], g_dt)
                    for kk in range(KEFF):
                        nc.tensor.matmul(
                            g[:],
                            lhsT=w_n,
                            rhs=ddv[:, kk * L + h * MMF : kk * L + (h + 1) * MMF],
                            start=(kk == 0),
                            stop=(kk == KEFF - 1),
                        )
                    if STAGE == "mm":
                        continue
                    prod = prod_pool.tile([D, MMF], prod_dt)
                    if HYB and h % HYB == 0:
                        # ACT evicts PSUM as f16 so the DVE multiply runs in
                        # its 2x packed mode — balances DVE vs ACT load
                        gs = gs_pool.tile([D, MMF], F16)
                        nc.scalar.copy(gs[:], g[:])
                        nc.vector.tensor_mul(prod[:], gs[:], embT16[:, ls])
                    else:
                        nc.vector.tensor_mul(prod[:], g[:], embT[:, ls])
                    if STAGE in ("red", "full"):
                        pending.append((n, grp, j, prod))
                # drain the PREVIOUS group's reduces so the PE never waits on
                # the DVE chain of the group it just issued
                while len(pending) > GRP or (
                    pending and (n, grp) == (NB - 1, NGRPS - 1)
                ):
                    flush = pending[:GRP]
                    del pending[:GRP]
                    row = row_psum.tile([128, MMF], g_dt, tag="row")
                    for pn, pgrp, pj, prod in flush:
                        # partition-reduce over d: ones stationary (32 dup
                        # cols); chunk pj lands at partitions [32pj, 32pj+32)
                        nc.tensor.matmul(
                            row[32 * pj : 32 * pj + 32, :], lhsT=ones[:],
                            rhs=prod[:], start=True, stop=True,
                            tile_position=(0, 32 * pj),
                        )
                    pn, pgrp = flush[0][0], flush[0][1]
                    gcol = (pn * NGRPS + pgrp) * MMF
                    nc.scalar.copy(out_sb[:, gcol : gcol + MMF], row[:])
                    done_groups[pn] = done_groups.get(pn, 0) + 1
                    if STAGE == "full" and done_groups[pn] == NGRPS:
                        # store batch pn on SWDGE so the HWDGE load FIFO
                        # never queues a store behind the next delta load
                        nc.gpsimd.dma_start(
                            out=outp[pn].rearrange(
                                "(grp j f) -> j grp f", j=GRP, f=MMF
                            ),
                            in_=out_sb[
                                :, pn * NGRPS * MMF : (pn + 1) * NGRPS * MMF
                            ].rearrange(
                                "(j d) (grp f) -> j d grp f", d=32, f=MMF
                            )[:, 0],
                        )

    nc.compile()
    return nc


_NC_CACHE: dict[str, bass.Bass] = {}


def _get_nc() -> bass.Bass:
    key = f"{LOOP_R}:{STAGE}:{DDBUFS}:{GDT}:{HYB}:{DENG}:{EMBENG}:{K1}"
    if key not in _NC_CACHE:
        _NC_CACHE[key] = _build_nc()
    return _NC_CACHE[key]


def prepare_in_maps(self_attn, self_delta, emb_table, value_w):
    """Host-side prep: cast/fold/transpose + shard over batch."""
    self_attn = np.asarray(self_attn, dtype=np.float32)
    self_delta = np.asarray(self_delta, dtype=np.float32)
    emb_table = np.asarray(emb_table, dtype=np.float32)
    value_w = np.asarray(value_w, dtype=np.float32)
    assert self_attn.shape == (N, M, D), self_attn.shape
    assert self_delta.shape == (N, M, L, K), self_delta.shape
    assert emb_table.shape == (L + 1, D), emb_table.shape

    if K1:
        # host k-sum: [N, M, L, K] f32 -> [N, M, L] bf16 -> f32 view
        dkl = (
            (self_delta[..., 0] + self_delta[..., 1])
            .astype(NPBF16)
            .view(np.float32)
            .reshape(N, M, L // 2)
        )
    else:
        # [N, M, L, K] f32 -> [N, M, K, L] bf16, then f32 view [N, M, K*L/2]
        dkl = np.ascontiguousarray(
            self_delta.astype(NPBF16).transpose(0, 1, 3, 2)
        ).view(np.float32).reshape(N, M, K * L // 2)
    w = (self_attn * value_w[None, :, None]).astype(NPBF16)  # [N, M, D]
    embt32 = np.ascontiguousarray(emb_table[1:].T)  # [D, L] f32
    embt = embt32.astype(NPBF16) if GDT == "bf16" else embt32

    in_maps = []
    for c in range(NCORES):
        n0 = c * NB
        m = {
            "delta": dkl[n0 : n0 + NB],
            "w": np.ascontiguousarray(
                w[n0 : n0 + NB].transpose(1, 0, 2)
            ).reshape(M, NB * D),
            "embt": embt,
        }
        if HYB:
            m["embt16"] = embt32.astype(np.float16)
        in_maps.append(m)
    return in_maps


def kernel(self_attn, self_delta, emb_table, value_w, traj_len=None, loc_max=None,
           _trace=False, _tmpdir=None):
    """Full inputs in, full output out.  traj_len is unused by the reference."""
    if loc_max is not None:
        assert int(loc_max) == L, loc_max

    in_maps = prepare_in_maps(self_attn, self_delta, emb_table, value_w)

    nc = _get_nc()
    try:
        res = run_bass_kernel_spmd(
            nc, in_maps, list(range(NCORES)), trace=_trace, tmpdir=_tmpdir
        )
    except Exception:
        # one retry for transient NRT execution failures
        res = run_bass_kernel_spmd(
            nc, in_maps, list(range(NCORES)), trace=_trace, tmpdir=_tmpdir
        )
    out = np.concatenate([res.results[c]["out"] for c in range(NCORES)], axis=0)
    if _trace:
        return out, res
    return out
